# revision 17
# baseline (speedup 1.0000x reference)
"""Multi-head attention (B=2, S=2048, dim=2048, H=16, D=128) on 8 TRN2 NeuronCores.

Strategy: tensor-parallel over heads for qkv-proj + attention (each core owns
2 heads for ALL tokens, so K/V never move between cores), then 8-core
AllToAlls (one per local head, overlapped with attention) redistribute the
per-head attention outputs to a per-token sharding, and each core runs the
output projection for its 512 tokens (no all-reduce).

Per-core bass program (SPMD, identical on all 8 cores):
  A) qkv proj: QT/KT [d, tokens] transposed + V [tokens, d] natural, bf16.
  B) attention per (head, batch): scoresT[k,q] = KT.T @ QT on PE, exp on ACT,
     PV on PE; raw attn evicted to SBUF, then normalized by 1/rowsum
     (DVE accumulate + GpSimd partition_all_reduce) off the critical path.
  C) AllToAll per head -> attn_all [hd, 512 tok]; out = attn_all.T @ WoutT.

Inputs are cast to bf16 on host; matmuls accumulate in fp32 PSUM; output fp32.
"""
import os
import numpy as np
import ml_dtypes

import concourse.bass as bass
import concourse.bacc as bacc
import concourse.tile as tile
import concourse.mybir as mybir
import concourse.bass_isa as bass_isa
from concourse.bass_utils import run_bass_kernel_spmd
from concourse.masks import make_identity

B, S, DIM, H, D = 2, 2048, 2048, 16, 128
NC_N = 8
T = B * S                 # 4096 tokens total
TOK = T // NC_N           # 512 tokens per core (out-proj shard)
HPC = H // NC_N           # 2 heads per core
SCALE = float(D) ** -0.5

BF = mybir.dt.bfloat16
F32 = mybir.dt.float32

_CACHE: dict = {}


def _build():
    nc = bacc.Bacc("TRN2", target_bir_lowering=False, debug=False, num_devices=NC_N)
    xT_ap = nc.dram_tensor("xT", [DIM, T], BF, kind="ExternalInput").ap()
    wT_ap = nc.dram_tensor("wT", [DIM, 3 * HPC * D], BF, kind="ExternalInput").ap()
    woT_ap = nc.dram_tensor("woT", [H * D, DIM], BF, kind="ExternalInput").ap()
    out_ap = nc.dram_tensor("out", [TOK, DIM], F32, kind="ExternalOutput").ap()

    P = 128
    DC = DIM // P            # 16 contraction chunks
    QCOL = HPC * D           # 256 q/k/v columns per core

    with tile.TileContext(nc) as tc:
        with tc.tile_pool(name="persist", bufs=1) as persist, \
             tc.tile_pool(name="dram", bufs=1, space="DRAM") as dram:

            # persistent SBUF tensors
            qt_sb = persist.tile([P, HPC, T], BF, tag="qt")      # Q^T
            kt_sb = persist.tile([P, HPC, T], BF, tag="kt")      # K^T
            vt_sb = persist.tile([P, HPC, T], BF, tag="vt")      # V^T
            attn_sb = persist.tile([P, HPC, T], BF, tag="attn")  # normalized attn^T
            ones_col = persist.tile([P, 1], BF, tag="onec")
            ones_row = persist.tile([1, P], BF, tag="oner")
            nc.vector.memset(ones_col[:], 1.0)
            nc.vector.memset(ones_row[:], 1.0)
            ident = persist.tile([P, P], BF, tag="ident")
            make_identity(nc, ident[:])

            # A2A bounce buffers, one pair per local head
            a2a_in = [dram.tile([NC_N * D, TOK], BF, tag=f"a2ain{h}", name=f"a2ain{h}")
                      for h in range(HPC)]
            a2a_out = [dram.tile([NC_N * D, TOK], BF, tag=f"a2aout{h}", name=f"a2aout{h}")
                       for h in range(HPC)]

            # ---- Stage A: qkv projection ----
            with tc.tile_pool(name="w", bufs=1) as wpool, \
                 tc.tile_pool(name="xin", bufs=6) as xpool, \
                 tc.tile_pool(name="psA", bufs=2, space="PSUM") as psA:
                w_sb = wpool.tile([P, DC, 3 * QCOL], BF)
                for wg in range(4):
                    nc.sync.dma_start(
                        out=w_sb[:, wg * 4:(wg + 1) * 4, :],
                        in_=wT_ap.rearrange("(dc p) c -> p dc c", p=P)[
                            :, wg * 4:(wg + 1) * 4, :])

                for t2 in range(T // 2048):      # 2 token chunks of 2048
                    xts = []
                    for q4 in range(4):
                        xh = xpool.tile([P, DC, 512], BF, tag="xt",
                                        name=f"xt{t2}_{q4}")
                        for wg in range(4):
                            nc.sync.dma_start(
                                out=xh[:, wg * 4:(wg + 1) * 4, :],
                                in_=xT_ap.rearrange("(dc p) n -> p dc n", p=P)[
                                    :, wg * 4:(wg + 1) * 4,
                                    t2 * 2048 + q4 * 512:
                                    t2 * 2048 + (q4 + 1) * 512])
                        xts.append(xh)
                    for oc in range(3 * HPC):    # Q0 Q1 K0 K1 V0 V1 (V^T)
                        ps = psA.tile([P, 2048], F32, tag="ps",
                                      name=f"psA{t2}_{oc}")
                        for dc in range(DC):
                            for q4 in range(4):
                                nc.tensor.matmul(
                                    ps[:, q4 * 512:(q4 + 1) * 512],
                                    w_sb[:, dc, oc * P:(oc + 1) * P],
                                    xts[q4][:, dc, :],
                                    start=(dc == 0), stop=(dc == DC - 1))
                        dst = (qt_sb, kt_sb, vt_sb)[oc // HPC]
                        hc = oc % HPC
                        nc.scalar.activation(
                            dst[:, hc, t2 * 2048:(t2 + 1) * 2048], ps[:],
                            mybir.ActivationFunctionType.Copy)
            # Wout^T, loaded during attention (own pool so its SBUF space
            # is disjoint from stage A's w/x pools)
            wop_cm = tc.tile_pool(name="wop", bufs=1)
            wopool = wop_cm.__enter__()
            wo_sb = wopool.tile([P, H * D // P, DIM], BF, tag="wo")
            nc.sync.dma_start(
                out=wo_sb[:], in_=woT_ap.rearrange("(hc p) d -> p hc d", p=P))

            # ---- Stage B: attention per (head, batch) + per-head A2A ----
            with tc.tile_pool(name="exp", bufs=8) as epool, \
                 tc.tile_pool(name="accp", bufs=2) as apool, \
                 tc.tile_pool(name="raw", bufs=2) as rawpool, \
                 tc.tile_pool(name="vun", bufs=2) as vun, \
                 tc.tile_pool(name="pss", bufs=2, space="PSUM") as pss, \
                 tc.tile_pool(name="psa", bufs=1, space="PSUM") as psa, \
                 tc.tile_pool(name="psd", bufs=1, space="PSUM") as psd, \
                 tc.tile_pool(name="psT", bufs=1, space="PSUM") as psT:
                KC = S // P   # 16 key chunks
                for h in range(HPC):
                    for b in range(B):
                        t0 = b * S
                        vunit = vun.tile([P, S // P, P], BF, tag="vu",
                                         name=f"vu{h}_{b}")
                        for kk in range(S // P):
                            tp = psT.tile([P, P], BF, tag="tp",
                                          name=f"tp{h}_{b}_{kk}")
                            nc.tensor.transpose(
                                tp[:],
                                vt_sb[:, h, t0 + kk * P: t0 + (kk + 1) * P],
                                ident[:])
                            nc.scalar.activation(
                                vunit[:, kk, :], tp[:],
                                mybir.ActivationFunctionType.Copy)
                        for qh in range(2):       # q halves of 1024
                            q0 = t0 + qh * 1024
                            ps_attn = psa.tile([P, 1024], F32, tag="psa")
                            acc2 = [apool.tile([P, 1024], F32, tag=f"acc{i}",
                                               name=f"acc{i}")
                                    for i in range(2)]
                            for kc in range(KC):
                                ps_s = pss.tile([P, 1024], F32, tag="pss")
                                kslice = kt_sb[:, h, t0 + kc * P: t0 + (kc + 1) * P]
                                for qs in range(2):
                                    nc.tensor.matmul(
                                        ps_s[:, qs * 512:(qs + 1) * 512],
                                        kslice,
                                        qt_sb[:, h, q0 + qs * 512: q0 + (qs + 1) * 512],
                                        start=True, stop=True)
                                et = epool.tile([P, 1024], BF, tag="exp")
                                nc.scalar.activation(
                                    et[:], ps_s[:],
                                    mybir.ActivationFunctionType.Exp, scale=SCALE)
                                accx = acc2[kc % 2]
                                if kc < 2:
                                    nc.vector.tensor_copy(out=accx[:], in_=et[:])
                                else:
                                    nc.vector.tensor_tensor(
                                        out=accx[:], in0=accx[:], in1=et[:],
                                        op=mybir.AluOpType.add)
                                vslice = vunit[:, kc, :]
                                for qs in range(2):
                                    nc.tensor.matmul(
                                        ps_attn[:, qs * 512:(qs + 1) * 512],
                                        vslice,
                                        et[:, qs * 512:(qs + 1) * 512],
                                        start=(kc == 0), stop=(kc == KC - 1))
                            # evict raw attn so PSUM frees without waiting on
                            # the normalization chain
                            araw = rawpool.tile([P, 1024], F32, tag="araw")
                            nc.scalar.activation(
                                araw[:], ps_attn[:],
                                mybir.ActivationFunctionType.Copy)
                            accb = apool.tile([P, 1024], BF, tag="accb")
                            nc.vector.tensor_tensor(
                                out=accb[:], in0=acc2[0][:], in1=acc2[1][:],
                                op=mybir.AluOpType.add)
                            for qs in range(2):
                                dn = psd.tile([1, 512], F32, tag="dnbc",
                                              name=f"dn{h}{b}{qh}{qs}")
                                nc.tensor.matmul(
                                    dn[:], ones_col[:],
                                    accb[:, qs * 512:(qs + 1) * 512],
                                    start=True, stop=True)
                                rd = apool.tile([1, 512], F32, tag="rd")
                                nc.vector.reciprocal_approx_fast(
                                    out=rd[:], in_=dn[:])
                                rdb = apool.tile([1, 512], BF, tag="rdb")
                                nc.vector.tensor_copy(out=rdb[:], in_=rd[:])
                                bc = psd.tile([P, 512], F32, tag="dnbc",
                                              name=f"bc{h}{b}{qh}{qs}")
                                nc.tensor.matmul(
                                    bc[:], ones_row[:], rdb[:],
                                    start=True, stop=True)
                                nc.vector.tensor_tensor(
                                    out=attn_sb[:, h,
                                                q0 + qs * 512:q0 + (qs + 1) * 512],
                                    in0=araw[:, qs * 512:(qs + 1) * 512],
                                    in1=bc[:],
                                    op=mybir.AluOpType.mult)
                        # stage this batch's token slices for the A2A now
                        for j in range(b * 4, (b + 1) * 4):
                            nc.sync.dma_start(
                                out=a2a_in[h][j * D:(j + 1) * D, :].rearrange(
                                    "(one p) f -> p one f", p=P),
                                in_=attn_sb[:, h:h + 1, j * TOK:(j + 1) * TOK])
                    # head fully staged on all cores at the same program
                    # point -> fire its AllToAll while the next head computes
                    nc.gpsimd.collective_compute(
                        "AllToAll", mybir.AluOpType.bypass,
                        replica_groups=[list(range(NC_N))],
                        ins=[a2a_in[h].opt()], outs=[a2a_out[h].opt()])

            # ---- Stage C: output projection ----
            with tc.tile_pool(name="attall", bufs=1) as allpool, \
                 tc.tile_pool(name="outp", bufs=3) as outpool, \
                 tc.tile_pool(name="psc", bufs=4, space="PSUM") as psc:
                # attn_all[h] rows i*128+p = global head (2i+h), dim p
                attn_all = [allpool.tile([P, NC_N, TOK], BF, tag=f"al{h}", name=f"al{h}")
                            for h in range(HPC)]
                for h in range(HPC):
                    nc.sync.dma_start(
                        out=attn_all[h][:],
                        in_=a2a_out[h].rearrange("(i p) f -> p i f", p=P))
                out_view = out_ap.rearrange("(qs p) d -> p qs d", p=P)
                for qs in range(TOK // P):       # 4
                    pss_c = [psc.tile([P, 512], F32, tag="psc",
                                      name=f"psc{qs}_{d_}")
                             for d_ in range(4)]
                    for h in range(HPC):         # accumulate h=0 heads first
                        for i in range(NC_N):
                            g = 2 * i + h        # global head = wo row chunk
                            first = (h == 0 and i == 0)
                            last = (h == HPC - 1 and i == NC_N - 1)
                            for ds in range(4):
                                nc.tensor.matmul(
                                    pss_c[ds][:],
                                    attn_all[h][:, i, qs * P:(qs + 1) * P],
                                    wo_sb[:, g, ds * 512:(ds + 1) * 512],
                                    start=first, stop=last)
                    for ds in range(4):
                        ot = outpool.tile([P, 512], F32, tag="ot")
                        nc.scalar.activation(
                            ot[:], pss_c[ds][:],
                            mybir.ActivationFunctionType.Copy)
                        nc.sync.dma_start(
                            out=out_view[:, qs, ds * 512:(ds + 1) * 512],
                            in_=ot[:])
            wop_cm.__exit__(None, None, None)

    nc.compile()
    return nc


def _get_nc():
    if "nc" not in _CACHE:
        if os.environ.get("KERNEL_TRACE"):
            try:
                import axon_profile_shim
                axon_profile_shim.install()
            except Exception:
                pass
        _CACHE["nc"] = _build()
    return _CACHE["nc"]


def kernel(x, Wqkv, Wout):
    nc = _get_nc()

    xb = np.asarray(x, np.float32).reshape(T, DIM)
    xT = np.ascontiguousarray(xb.T).astype(ml_dtypes.bfloat16)
    Wqkv = np.asarray(Wqkv, np.float32)
    woT = np.ascontiguousarray(np.asarray(Wout, np.float32).T).astype(
        ml_dtypes.bfloat16)

    in_maps = []
    for c in range(NC_N):
        wq = Wqkv[HPC * D * c: HPC * D * (c + 1)]
        wk = Wqkv[H * D + HPC * D * c: H * D + HPC * D * (c + 1)]
        wv = Wqkv[2 * H * D + HPC * D * c: 2 * H * D + HPC * D * (c + 1)]
        wT = np.ascontiguousarray(
            np.concatenate([wq, wk, wv], axis=0).T).astype(ml_dtypes.bfloat16)
        in_maps.append({"xT": xT, "wT": wT, "woT": woT})

    trace = bool(os.environ.get("KERNEL_TRACE"))
    res = run_bass_kernel_spmd(
        nc, in_maps, core_ids=list(range(NC_N)), trace=trace)
    _CACHE["exec_time_ns"] = res.exec_time_ns

    out = np.concatenate(
        [res.results[c]["out"] for c in range(NC_N)], axis=0)
    return out.reshape(B, S, DIM).astype(np.float32)


# revision 19
# speedup vs baseline: 1.0465x; 1.0465x over previous
"""Multi-head attention (B=2, S=2048, dim=2048, H=16, D=128) on 8 TRN2 NeuronCores.

Strategy: tensor-parallel over heads for qkv-proj + attention (each core owns
2 heads for ALL tokens, so K/V never move between cores), then 8-core
AllToAlls (one per local head, overlapped with attention) redistribute the
per-head attention outputs to a per-token sharding, and each core runs the
output projection for its 512 tokens (no all-reduce).

Per-core bass program (SPMD, identical on all 8 cores):
  A) qkv proj: QT/KT [d, tokens] transposed + V [tokens, d] natural, bf16.
  B) attention per (head, batch): scoresT[k,q] = KT.T @ QT on PE, exp on ACT,
     PV on PE; raw attn evicted to SBUF, then normalized by 1/rowsum
     (DVE accumulate + GpSimd partition_all_reduce) off the critical path.
  C) AllToAll per head -> attn_all [hd, 512 tok]; out = attn_all.T @ WoutT.

Inputs are cast to bf16 on host; matmuls accumulate in fp32 PSUM; output fp32.
"""
import os
import numpy as np
import ml_dtypes

import concourse.bass as bass
import concourse.bacc as bacc
import concourse.tile as tile
import concourse.mybir as mybir
import concourse.bass_isa as bass_isa
from concourse.bass_utils import run_bass_kernel_spmd
from concourse.masks import make_identity

B, S, DIM, H, D = 2, 2048, 2048, 16, 128
NC_N = 8
T = B * S                 # 4096 tokens total
TOK = T // NC_N           # 512 tokens per core (out-proj shard)
HPC = H // NC_N           # 2 heads per core
SCALE = float(D) ** -0.5

BF = mybir.dt.bfloat16
F32 = mybir.dt.float32

_CACHE: dict = {}


def _build():
    nc = bacc.Bacc("TRN2", target_bir_lowering=False, debug=False, num_devices=NC_N)
    xT_ap = nc.dram_tensor("xT", [DIM, T], BF, kind="ExternalInput").ap()
    wT_ap = nc.dram_tensor("wT", [DIM, 3 * HPC * D], BF, kind="ExternalInput").ap()
    woT_ap = nc.dram_tensor("woT", [H * D, DIM], BF, kind="ExternalInput").ap()
    out_ap = nc.dram_tensor("out", [TOK, DIM], F32, kind="ExternalOutput").ap()

    P = 128
    DC = DIM // P            # 16 contraction chunks
    QCOL = HPC * D           # 256 q/k/v columns per core

    with tile.TileContext(nc) as tc:
        with tc.tile_pool(name="persist", bufs=1) as persist, \
             tc.tile_pool(name="dram", bufs=1, space="DRAM") as dram:

            # persistent SBUF tensors
            qt_sb = persist.tile([P, HPC, T], BF, tag="qt")      # Q^T
            kt_sb = persist.tile([P, HPC, T], BF, tag="kt")      # K^T
            vt_sb = persist.tile([P, HPC, T], BF, tag="vt")      # V^T
            attn_sb = persist.tile([P, HPC, T], BF, tag="attn")  # normalized attn^T
            ones_col = persist.tile([P, 1], BF, tag="onec")
            ones_row = persist.tile([1, P], BF, tag="oner")
            nc.vector.memset(ones_col[:], 1.0)
            nc.vector.memset(ones_row[:], 1.0)
            ident = persist.tile([P, P], BF, tag="ident")
            make_identity(nc, ident[:])

            # A2A bounce buffers, one pair per local head
            a2a_in = [dram.tile([NC_N * D, TOK], BF, tag=f"a2ain{h}", name=f"a2ain{h}")
                      for h in range(HPC)]
            a2a_out = [dram.tile([NC_N * D, TOK], BF, tag=f"a2aout{h}", name=f"a2aout{h}")
                       for h in range(HPC)]

            # ---- Stage A: qkv projection ----
            with tc.tile_pool(name="w", bufs=1) as wpool, \
                 tc.tile_pool(name="xin", bufs=6) as xpool, \
                 tc.tile_pool(name="psA", bufs=2, space="PSUM") as psA:
                w_sb = wpool.tile([P, DC, 3 * QCOL], BF)
                for wg in range(4):
                    (nc.sync, nc.scalar, nc.sync, nc.gpsimd)[wg].dma_start(
                        out=w_sb[:, wg * 4:(wg + 1) * 4, :],
                        in_=wT_ap.rearrange("(dc p) c -> p dc c", p=P)[
                            :, wg * 4:(wg + 1) * 4, :])

                for t2 in range(T // 2048):      # 2 token chunks of 2048
                    xts = []
                    for q4 in range(4):
                        xh = xpool.tile([P, DC, 512], BF, tag="xt",
                                        name=f"xt{t2}_{q4}")
                        engs = (nc.sync, nc.scalar, nc.sync, nc.gpsimd)
                        for wg in range(4):
                            engs[wg].dma_start(
                                out=xh[:, wg * 4:(wg + 1) * 4, :],
                                in_=xT_ap.rearrange("(dc p) n -> p dc n", p=P)[
                                    :, wg * 4:(wg + 1) * 4,
                                    t2 * 2048 + q4 * 512:
                                    t2 * 2048 + (q4 + 1) * 512])
                        xts.append(xh)
                    for oc in range(3 * HPC):    # Q0 Q1 K0 K1 V0 V1 (V^T)
                        ps = psA.tile([P, 2048], F32, tag="ps",
                                      name=f"psA{t2}_{oc}")
                        for dc in range(DC):
                            for q4 in range(4):
                                nc.tensor.matmul(
                                    ps[:, q4 * 512:(q4 + 1) * 512],
                                    w_sb[:, dc, oc * P:(oc + 1) * P],
                                    xts[q4][:, dc, :],
                                    start=(dc == 0), stop=(dc == DC - 1))
                        dst = (qt_sb, kt_sb, vt_sb)[oc // HPC]
                        hc = oc % HPC
                        nc.scalar.activation(
                            dst[:, hc, t2 * 2048:(t2 + 1) * 2048], ps[:],
                            mybir.ActivationFunctionType.Copy)
            # Wout^T, loaded during attention (own pool so its SBUF space
            # is disjoint from stage A's w/x pools)
            wop_cm = tc.tile_pool(name="wop", bufs=1)
            wopool = wop_cm.__enter__()
            wo_sb = wopool.tile([P, H * D // P, DIM], BF, tag="wo")
            nc.sync.dma_start(
                out=wo_sb[:], in_=woT_ap.rearrange("(hc p) d -> p hc d", p=P))

            # ---- Stage B: attention per (head, batch) + per-head A2A ----
            with tc.tile_pool(name="exp", bufs=8) as epool, \
                 tc.tile_pool(name="accp", bufs=2) as apool, \
                 tc.tile_pool(name="raw", bufs=2) as rawpool, \
                 tc.tile_pool(name="vun", bufs=2) as vun, \
                 tc.tile_pool(name="pss", bufs=2, space="PSUM") as pss, \
                 tc.tile_pool(name="psa", bufs=1, space="PSUM") as psa, \
                 tc.tile_pool(name="psd", bufs=1, space="PSUM") as psd, \
                 tc.tile_pool(name="psT", bufs=1, space="PSUM") as psT:
                KC = S // P   # 16 key chunks
                for h in range(HPC):
                    for b in range(B):
                        t0 = b * S
                        vunit = vun.tile([P, S // P, P], BF, tag="vu",
                                         name=f"vu{h}_{b}")
                        for kk in range(S // P):
                            tp = psT.tile([P, P], BF, tag="tp",
                                          name=f"tp{h}_{b}_{kk}")
                            nc.tensor.transpose(
                                tp[:],
                                vt_sb[:, h, t0 + kk * P: t0 + (kk + 1) * P],
                                ident[:])
                            nc.scalar.activation(
                                vunit[:, kk, :], tp[:],
                                mybir.ActivationFunctionType.Copy)
                        for qh in range(2):       # q halves of 1024
                            q0 = t0 + qh * 1024
                            ps_attn = psa.tile([P, 1024], F32, tag="psa")
                            acc2 = [apool.tile([P, 1024], F32, tag=f"acc{i}",
                                               name=f"acc{i}")
                                    for i in range(2)]
                            for kc in range(KC):
                                ps_s = pss.tile([P, 1024], F32, tag="pss")
                                kslice = kt_sb[:, h, t0 + kc * P: t0 + (kc + 1) * P]
                                for qs in range(2):
                                    nc.tensor.matmul(
                                        ps_s[:, qs * 512:(qs + 1) * 512],
                                        kslice,
                                        qt_sb[:, h, q0 + qs * 512: q0 + (qs + 1) * 512],
                                        start=True, stop=True)
                                et = epool.tile([P, 1024], BF, tag="exp")
                                nc.scalar.activation(
                                    et[:], ps_s[:],
                                    mybir.ActivationFunctionType.Exp, scale=SCALE)
                                accx = acc2[kc % 2]
                                if kc < 2:
                                    nc.vector.tensor_copy(out=accx[:], in_=et[:])
                                else:
                                    nc.vector.tensor_tensor(
                                        out=accx[:], in0=accx[:], in1=et[:],
                                        op=mybir.AluOpType.add)
                                vslice = vunit[:, kc, :]
                                for qs in range(2):
                                    nc.tensor.matmul(
                                        ps_attn[:, qs * 512:(qs + 1) * 512],
                                        vslice,
                                        et[:, qs * 512:(qs + 1) * 512],
                                        start=(kc == 0), stop=(kc == KC - 1))
                            # evict raw attn so PSUM frees without waiting on
                            # the normalization chain
                            araw = rawpool.tile([P, 1024], F32, tag="araw")
                            nc.scalar.activation(
                                araw[:], ps_attn[:],
                                mybir.ActivationFunctionType.Copy)
                            accb = apool.tile([P, 1024], BF, tag="accb")
                            nc.vector.tensor_tensor(
                                out=accb[:], in0=acc2[0][:], in1=acc2[1][:],
                                op=mybir.AluOpType.add)
                            for qs in range(2):
                                dn = psd.tile([1, 512], F32, tag="dnbc",
                                              name=f"dn{h}{b}{qh}{qs}")
                                nc.tensor.matmul(
                                    dn[:], ones_col[:],
                                    accb[:, qs * 512:(qs + 1) * 512],
                                    start=True, stop=True)
                                rd = apool.tile([1, 512], F32, tag="rd")
                                nc.vector.reciprocal_approx_fast(
                                    out=rd[:], in_=dn[:])
                                rdb = apool.tile([1, 512], BF, tag="rdb")
                                nc.vector.tensor_copy(out=rdb[:], in_=rd[:])
                                bc = psd.tile([P, 512], F32, tag="dnbc",
                                              name=f"bc{h}{b}{qh}{qs}")
                                nc.tensor.matmul(
                                    bc[:], ones_row[:], rdb[:],
                                    start=True, stop=True)
                                nc.vector.tensor_tensor(
                                    out=attn_sb[:, h,
                                                q0 + qs * 512:q0 + (qs + 1) * 512],
                                    in0=araw[:, qs * 512:(qs + 1) * 512],
                                    in1=bc[:],
                                    op=mybir.AluOpType.mult)
                        # stage this batch's token slices for the A2A now
                        for j in range(b * 4, (b + 1) * 4):
                            nc.sync.dma_start(
                                out=a2a_in[h][j * D:(j + 1) * D, :].rearrange(
                                    "(one p) f -> p one f", p=P),
                                in_=attn_sb[:, h:h + 1, j * TOK:(j + 1) * TOK])
                    # head fully staged on all cores at the same program
                    # point -> fire its AllToAll while the next head computes
                    nc.gpsimd.collective_compute(
                        "AllToAll", mybir.AluOpType.bypass,
                        replica_groups=[list(range(NC_N))],
                        ins=[a2a_in[h].opt()], outs=[a2a_out[h].opt()])

            # ---- Stage C: output projection ----
            with tc.tile_pool(name="attall", bufs=1) as allpool, \
                 tc.tile_pool(name="oacc", bufs=1) as oaccpool, \
                 tc.tile_pool(name="outp", bufs=4) as outpool, \
                 tc.tile_pool(name="psc", bufs=4, space="PSUM") as psc:
                # attn_all[h] rows i*128+p = global head (2i+h), dim p
                attn_all = [allpool.tile([P, NC_N, TOK], BF, tag=f"al{h}",
                                         name=f"al{h}")
                            for h in range(HPC)]
                for h in range(HPC):
                    nc.gpsimd.dma_start(
                        out=attn_all[h][:],
                        in_=a2a_out[h].rearrange("(i p) f -> p i f", p=P))
                out_view = out_ap.rearrange("(qs p) d -> p qs d", p=P)
                oacc = oaccpool.tile([P, TOK // P, DIM], F32, tag="oacc")
                # pass 1: h=0 heads (available right after the first A2A)
                for qs in range(TOK // P):       # 4
                    for ds in range(4):
                        ps = psc.tile([P, 512], F32, tag="psc",
                                      name=f"psc0_{qs}_{ds}")
                        for i in range(NC_N):
                            nc.tensor.matmul(
                                ps[:],
                                attn_all[0][:, i, qs * P:(qs + 1) * P],
                                wo_sb[:, 2 * i, ds * 512:(ds + 1) * 512],
                                start=(i == 0), stop=(i == NC_N - 1))
                        nc.scalar.activation(
                            oacc[:, qs, ds * 512:(ds + 1) * 512], ps[:],
                            mybir.ActivationFunctionType.Copy)
                # pass 2: h=1 heads, add pass-1 partial, write out
                for qs in range(TOK // P):
                    for ds in range(4):
                        ps = psc.tile([P, 512], F32, tag="psc",
                                      name=f"psc1_{qs}_{ds}")
                        for i in range(NC_N):
                            nc.tensor.matmul(
                                ps[:],
                                attn_all[1][:, i, qs * P:(qs + 1) * P],
                                wo_sb[:, 2 * i + 1, ds * 512:(ds + 1) * 512],
                                start=(i == 0), stop=(i == NC_N - 1))
                        ot = outpool.tile([P, 512], F32, tag="ot")
                        nc.vector.tensor_tensor(
                            out=ot[:], in0=ps[:],
                            in1=oacc[:, qs, ds * 512:(ds + 1) * 512],
                            op=mybir.AluOpType.add)
                        nc.sync.dma_start(
                            out=out_view[:, qs, ds * 512:(ds + 1) * 512],
                            in_=ot[:])
            wop_cm.__exit__(None, None, None)

    nc.compile()
    return nc


def _get_nc():
    if "nc" not in _CACHE:
        if os.environ.get("KERNEL_TRACE"):
            try:
                import axon_profile_shim
                axon_profile_shim.install()
            except Exception:
                pass
        _CACHE["nc"] = _build()
    return _CACHE["nc"]


def kernel(x, Wqkv, Wout):
    nc = _get_nc()

    xb = np.asarray(x, np.float32).reshape(T, DIM)
    xT = np.ascontiguousarray(xb.T).astype(ml_dtypes.bfloat16)
    Wqkv = np.asarray(Wqkv, np.float32)
    woT = np.ascontiguousarray(np.asarray(Wout, np.float32).T).astype(
        ml_dtypes.bfloat16)

    in_maps = []
    for c in range(NC_N):
        wq = Wqkv[HPC * D * c: HPC * D * (c + 1)]
        wk = Wqkv[H * D + HPC * D * c: H * D + HPC * D * (c + 1)]
        wv = Wqkv[2 * H * D + HPC * D * c: 2 * H * D + HPC * D * (c + 1)]
        wT = np.ascontiguousarray(
            np.concatenate([wq, wk, wv], axis=0).T).astype(ml_dtypes.bfloat16)
        in_maps.append({"xT": xT, "wT": wT, "woT": woT})

    trace = bool(os.environ.get("KERNEL_TRACE"))
    res = run_bass_kernel_spmd(
        nc, in_maps, core_ids=list(range(NC_N)), trace=trace)
    _CACHE["exec_time_ns"] = res.exec_time_ns

    out = np.concatenate(
        [res.results[c]["out"] for c in range(NC_N)], axis=0)
    return out.reshape(B, S, DIM).astype(np.float32)
